# revision 16
# baseline (speedup 1.0000x reference)
"""Trainium2 Bass kernel: single-head attention (B=4, S=2048, D=1024) on 8 NeuronCores.

Sharding: data-parallel over (batch, query-half): core c handles batch c//2,
query rows [c%2*1024, (c%2+1)*1024), and (dist mode) computes the K-projection
only for its own kv half; the pair {2b, 2b+1} exchanges K^T via a 2-rank
AllGather overlapped with the Q-projection and the local-half attention.

Math per core (all matmuls bf16, fp32 PSUM accumulation):
  QT[dk,q]  = Wq(lhsT) . XqT(rhs)                 (+bq)
  KT[dk,s]  = Wk(lhsT) . XkvT(rhs)                (+bk)   [local half, AG for rest]
  sT[s,q]   = KT-tiles(lhsT) . QT(rhs)            scores^T
  eT[s,q]   = exp(sT / sqrt(dk) [+ maskT])        ScalarE, PSUM->SBUF bf16
  sums[1,q] = ones(lhsT) . eT(rhs)                softmax denominators
  HT[dm,q]  = Xkv-tiles(lhsT) . eT(rhs)           H = probs_unnorm @ Xkv
  out[q,dv] = HT-tiles(lhsT) . Wv(rhs)  (+bv)     (probs@Xkv)@Wv == probs@(Xkv@Wv)
  out      *= 1/sums  (per-partition scale on ScalarE, fused with PSUM->SBUF copy)

dist mode details: the s axis lives in LOCAL-relative order on each core
(m-tiles 0..7 = own kv half, 8..15 = partner's). The host feeds xkv/maskt
pre-permuted to match. The AllGather output is rank-ordered (identical layout
on both cores), so the partner block is recovered SPMD-uniformly via the exact
bf16-bit identity  remote = g0 XOR g1 XOR local  on uint32 views (own block
round-trips bit-identically through the collective).
"""

import os
import numpy as np
import ml_dtypes

B, S, D = 4, 2048, 1024
N_CORES = 8
QL = S // 2  # query rows per core (1024)
BF16 = ml_dtypes.bfloat16

_cache: dict = {}


def _build(dist: bool, with_mask: bool, with_bq: bool, with_bk: bool,
           with_bv: bool):
    import concourse.bass as bass
    import concourse.mybir as mybir
    import concourse.tile as tile
    from concourse import bacc

    fp32 = mybir.dt.float32
    bf16 = mybir.dt.bfloat16
    uint32 = mybir.dt.uint32

    nc = bacc.Bacc("TRN2", target_bir_lowering=False, debug=False,
                   num_devices=N_CORES)

    KVL = QL if dist else S  # kv rows projected locally
    xqt_d = nc.dram_tensor("xqt", (D, QL), bf16, kind="ExternalInput")
    xkvt_d = nc.dram_tensor("xkvt", (D, KVL), bf16, kind="ExternalInput")
    xkv_d = nc.dram_tensor("xkv", (S, D), bf16, kind="ExternalInput")
    wq_d = nc.dram_tensor("wq", (D, D), bf16, kind="ExternalInput")
    wk_d = nc.dram_tensor("wk", (D, D), bf16, kind="ExternalInput")
    wv_d = nc.dram_tensor("wv", (D, D), bf16, kind="ExternalInput")
    if with_bq:
        bq_d = nc.dram_tensor("bq", (128, 8), fp32, kind="ExternalInput")
    if with_bk:
        bk_d = nc.dram_tensor("bk", (128, 8), fp32, kind="ExternalInput")
    if with_bv:
        bv_d = nc.dram_tensor("bv", (1, D), bf16, kind="ExternalInput")
    if with_mask:
        maskt_d = nc.dram_tensor("maskt", (S, QL), fp32, kind="ExternalInput")
    out_d = nc.dram_tensor("out", (QL, D), fp32, kind="ExternalOutput")

    if dist:
        cc_ins = [nc.dram_tensor(f"cc_in{x}", (D // 2, QL), bf16)
                  for x in range(2)]
        cc_outs = [nc.dram_tensor(f"cc_out{x}", (D, QL), bf16)
                   for x in range(2)]
        groups = [[2 * g, 2 * g + 1] for g in range(4)]

    NT_D = D // 128    # 8 tiles along d_model / d_key
    NT_S = S // 128    # 16 tiles along s
    NT_L = NT_S // 2   # 8 (local-half s tiles in dist mode)
    NQ = QL // 512     # 2 query chunks of 512
    NKV = KVL // 512   # local kv chunks
    NV = D // 512      # 2 dv chunks of 512
    SCALE = 1.0 / float(np.sqrt(D))

    with tile.TileContext(nc) as tc:
        with (
            tc.tile_pool(name="cons", bufs=1) as cons,
            tc.tile_pool(name="a2", bufs=4) as a2,
            tc.tile_pool(name="kt", bufs=1) as ktp,
            tc.tile_pool(name="et", bufs=1) as etp,
            tc.tile_pool(name="xq", bufs=2) as xqp,
            tc.tile_pool(name="xkvt", bufs=2 if dist else 3) as xkvtp,
            tc.tile_pool(name="xkv", bufs=2) as xkvp,
            tc.tile_pool(name="gch", bufs=2) as gchp,
            tc.tile_pool(name="outp", bufs=3) as outp,
            tc.tile_pool(name="mask", bufs=2) as maskp,
            tc.tile_pool(name="ps", bufs=4, space=bass.MemorySpace.PSUM) as psp,
            tc.tile_pool(name="pss", bufs=2, space=bass.MemorySpace.PSUM) as pssp,
            tc.tile_pool(name="pst", bufs=1, space=bass.MemorySpace.PSUM) as pstp,
        ):
            # ---- constants ----
            ones_col = cons.tile([128, 1], bf16, tag="ones_col")
            nc.gpsimd.memset(ones_col[:], 1.0)
            ident1 = cons.tile([1, 1], fp32, tag="ident1")
            nc.gpsimd.memset(ident1[:], 1.0)
            if with_bv:
                ones_row = cons.tile([1, 128], bf16, tag="ones_row")
                nc.gpsimd.memset(ones_row[:], 1.0)
                bv_sb = cons.tile([1, D], bf16, tag="bv")
                nc.sync.dma_start(bv_sb[:], bv_d.ap()[:])
            if with_bq:
                bq_sb = cons.tile([128, 8], fp32, tag="bq")
                nc.sync.dma_start(bq_sb[:], bq_d.ap()[:])
            if with_bk:
                bk_sb = cons.tile([128, 8], fp32, tag="bk")
                nc.sync.dma_start(bk_sb[:], bk_d.ap()[:])

            # ---- weight + first-activation loads; issue order matters: the
            # first PE group needs wk + xkvt chunk 0, so those DMAs go first.
            wk_sb = a2.tile([128, NT_D, D], bf16, tag="a2")
            for j in range(NT_D):
                nc.sync.dma_start(wk_sb[:, j, :],
                                  wk_d.ap()[j * 128:(j + 1) * 128, :])

            qt_sb = a2.tile([128, NT_D, QL], bf16, tag="a2")
            kt_sb = ktp.tile([128, NT_D, KVL], bf16, tag="kt")
            if dist:
                ktr_sb = ktp.tile([128, NT_D, QL], bf16, tag="ktr")
            et_sb = etp.tile([128, NT_S, QL], bf16, tag="et")

            def proj_group(ps, w_sb, x_ch, i):
                for j in range(NT_D):
                    nc.tensor.matmul(
                        ps[:], w_sb[:, j, i * 128:(i + 1) * 128], x_ch[:, j, :],
                        start=(j == 0), stop=(j == NT_D - 1))

            # ---- stage 1b: KT (local half in dist mode) -> kt_sb ----
            kx_chunks = []
            for n in range(NKV):
                xkvt_ch = xkvtp.tile([128, NT_D, 512], bf16, tag="xkvt")
                for j in range(NT_D):
                    nc.sync.dma_start(
                        xkvt_ch[:, j, :],
                        xkvt_d.ap()[j * 128:(j + 1) * 128,
                                    n * 512:(n + 1) * 512])
                kx_chunks.append(xkvt_ch)
            for i in range(NT_D):
                for n in range(NKV):
                    ps = psp.tile([128, 512], fp32, tag="ps")
                    proj_group(ps, wk_sb, kx_chunks[n], i)
                    if with_bk:
                        nc.scalar.activation(
                            kt_sb[:, i, n * 512:(n + 1) * 512], ps[:],
                            mybir.ActivationFunctionType.Identity,
                            bias=bk_sb[:, i:i + 1])
                    else:
                        nc.scalar.activation(
                            kt_sb[:, i, n * 512:(n + 1) * 512], ps[:],
                            mybir.ActivationFunctionType.Copy)
                if dist:
                    x, ir = divmod(i, NT_D // 2)
                    nc.sync.dma_start(
                        cc_ins[x].ap()[ir * 128:(ir + 1) * 128, :],
                        kt_sb[:, i, 0:QL])
                    # fire each half-AllGather as soon as its rows are staged,
                    # overlapping comm with the rest of the K projection
                    if ir == NT_D // 2 - 1:
                        nc.gpsimd.collective_compute(
                            "AllGather", mybir.AluOpType.bypass,
                            replica_groups=groups,
                            ins=[cc_ins[x].ap()[:].opt()],
                            outs=[cc_outs[x].ap()[:].opt()],
                        )

            # ---- stage 1a: QT -> qt_sb ----
            wq_sb = a2.tile([128, NT_D, D], bf16, tag="a2")
            for j in range(NT_D):
                nc.sync.dma_start(wq_sb[:, j, :],
                                  wq_d.ap()[j * 128:(j + 1) * 128, :])
            for n in range(NQ):
                xq_ch = xqp.tile([128, NT_D, 512], bf16, tag="xq")
                for j in range(NT_D):
                    nc.sync.dma_start(
                        xq_ch[:, j, :],
                        xqt_d.ap()[j * 128:(j + 1) * 128, n * 512:(n + 1) * 512])
                for i in range(NT_D):
                    ps = psp.tile([128, 512], fp32, tag="ps")
                    proj_group(ps, wq_sb, xq_ch, i)
                    if with_bq:
                        nc.scalar.activation(
                            qt_sb[:, i, n * 512:(n + 1) * 512], ps[:],
                            mybir.ActivationFunctionType.Identity,
                            bias=bq_sb[:, i:i + 1])
                    else:
                        nc.scalar.activation(
                            qt_sb[:, i, n * 512:(n + 1) * 512], ps[:],
                            mybir.ActivationFunctionType.Copy)

            # wv load early: reuses wk's pool slot once the K projection ends,
            # so stage 4 never waits on this DMA
            wv_sb = a2.tile([128, NT_D, D], bf16, tag="a2")
            for j in range(NT_D):
                nc.sync.dma_start(wv_sb[:, j, :],
                                  wv_d.ap()[j * 128:(j + 1) * 128, :])

            if dist:
                # read the gathered pair back, recover the partner's block via
                # remote = g0 ^ g1 ^ local (exact bf16 bit identity) -> ktr_sb
                NH = NT_D // 2
                for x in range(2):
                    for c in range(2):
                        g_ch = gchp.tile([128, 2 * NH, 512], bf16, tag="gch")
                        nc.sync.dma_start(
                            g_ch[:],
                            cc_outs[x].ap()[:, c * 512:(c + 1) * 512]
                            .rearrange("(g p) f -> p g f", p=128))
                        nc.vector.tensor_tensor(
                            g_ch[:, 0:NH, :].bitcast(uint32),
                            g_ch[:, 0:NH, :].bitcast(uint32),
                            g_ch[:, NH:2 * NH, :].bitcast(uint32),
                            mybir.AluOpType.bitwise_xor)
                        nc.vector.tensor_tensor(
                            ktr_sb[:, x * NH:(x + 1) * NH,
                                   c * 512:(c + 1) * 512].bitcast(uint32),
                            g_ch[:, 0:NH, :].bitcast(uint32),
                            kt_sb[:, x * NH:(x + 1) * NH,
                                  c * 512:(c + 1) * 512].bitcast(uint32),
                            mybir.AluOpType.bitwise_xor)

            # ---- stage 2: scores^T + exp ----
            def score_group(m, n):
                kt, mm = (ktr_sb, m - NT_L) if (dist and m >= NT_L) else (kt_sb, m)
                ps = psp.tile([128, 512], fp32, tag="ps")
                for i in range(NT_D):
                    nc.tensor.matmul(
                        ps[:], kt[:, i, mm * 128:(mm + 1) * 128],
                        qt_sb[:, i, n * 512:(n + 1) * 512],
                        start=(i == 0), stop=(i == NT_D - 1))
                if with_mask:
                    mk = maskp.tile([128, 512], fp32, tag="mask")
                    nc.sync.dma_start(
                        mk[:], maskt_d.ap()[m * 128:(m + 1) * 128,
                                            n * 512:(n + 1) * 512])
                    nc.vector.tensor_tensor(
                        ps[:], ps[:], mk[:], mybir.AluOpType.add)
                nc.scalar.activation(
                    et_sb[:, m, n * 512:(n + 1) * 512], ps[:],
                    mybir.ActivationFunctionType.Exp, scale=SCALE)

            first_ms = range(NT_L) if dist else range(NT_S)
            for n in range(NQ):
                for m in first_ms:
                    score_group(m, n)

            # ---- stage 3a: HT over available s-tiles ----
            ht_sb = a2.tile([128, NT_D, QL], bf16, tag="a2")
            xkv_chunks = {}

            def ht_groups(ms, merge):
                for j in range(NT_D):
                    if (j, ms[0]) not in xkv_chunks:
                        xkv_ch = xkvp.tile([128, len(ms), 128], bf16, tag="xkv")
                        nc.sync.dma_start(
                            xkv_ch[:],
                            xkv_d.ap()[ms[0] * 128:(ms[-1] + 1) * 128,
                                       j * 128:(j + 1) * 128]
                            .rearrange("(m p) d -> p m d", p=128))
                        xkv_chunks[(j, ms[0])] = xkv_ch
                    xkv_ch = xkv_chunks[(j, ms[0])]
                    for n in range(NQ):
                        ps = psp.tile([128, 512], fp32, tag="ps")
                        for k, m in enumerate(ms):
                            nc.tensor.matmul(
                                ps[:], xkv_ch[:, k, :],
                                et_sb[:, m, n * 512:(n + 1) * 512],
                                start=(k == 0), stop=(k == len(ms) - 1))
                        dst = ht_sb[:, j, n * 512:(n + 1) * 512]
                        if merge:
                            nc.vector.tensor_tensor(
                                dst, ps[:], dst, mybir.AluOpType.add)
                        else:
                            nc.scalar.activation(
                                dst, ps[:],
                                mybir.ActivationFunctionType.Copy)

            if dist:
                ht_groups(list(range(NT_L)), merge=False)
                for n in range(NQ):
                    for m in range(NT_L, NT_S):
                        score_group(m, n)
            else:
                ht_groups(list(range(NT_S)), merge=False)

            # sums[1, q] = sum_s eT (all s-tiles)
            sums_sb = cons.tile([1, QL], fp32, tag="sums")
            for n in range(NQ):
                pss = pssp.tile([1, 512], fp32, tag="pss")
                for m in range(NT_S):
                    nc.tensor.matmul(
                        pss[:], ones_col[:], et_sb[:, m, n * 512:(n + 1) * 512],
                        start=(m == 0), stop=(m == NT_S - 1))
                nc.scalar.activation(
                    sums_sb[:, n * 512:(n + 1) * 512], pss[:],
                    mybir.ActivationFunctionType.Copy)

            if dist:
                ht_groups(list(range(NT_L, NT_S)), merge=True)

            # transpose sums -> [q(part), 1]; reciprocal
            pst = pstp.tile([128, 8], fp32, tag="pst")
            for p in range(8):
                nc.tensor.transpose(
                    pst[:, p:p + 1], sums_sb[:, p * 128:(p + 1) * 128],
                    ident1[:])
            recip_sb = cons.tile([128, 8], fp32, tag="recip")
            nc.vector.reciprocal(recip_sb[:], pst[:])

            # ---- stage 4: out = HT^T . Wv (+bv), normalized ----
            for p in range(8):
                out_sb = outp.tile([128, D], fp32, tag="outsb")
                for n2 in range(NV):
                    ps = psp.tile([128, 512], fp32, tag="ps")
                    for j in range(NT_D):
                        nc.tensor.matmul(
                            ps[:], ht_sb[:, j, p * 128:(p + 1) * 128],
                            wv_sb[:, j, n2 * 512:(n2 + 1) * 512],
                            start=(j == 0),
                            stop=(j == NT_D - 1 and not with_bv))
                    if with_bv:
                        nc.tensor.matmul(
                            ps[:], ones_row[:],
                            bv_sb[:, n2 * 512:(n2 + 1) * 512],
                            start=False, stop=True)
                    nc.scalar.activation(
                        out_sb[:, n2 * 512:(n2 + 1) * 512], ps[:],
                        mybir.ActivationFunctionType.Copy,
                        scale=recip_sb[:, p:p + 1])
                nc.sync.dma_start(
                    out_d.ap()[p * 128:(p + 1) * 128, :], out_sb[:])

    nc.compile()
    return nc


def _get_nc(flags):
    if flags not in _cache:
        _cache[flags] = _build(*flags)
    return _cache[flags]


def _prep_in_maps(query_input, keyvalue_input, mask, Wq, bq, Wk, bk, Wv, bv,
                  dist=True):
    qi = np.asarray(query_input, np.float32)
    kv = np.asarray(keyvalue_input, np.float32)
    mask = np.asarray(mask, np.float32)
    Wqb = np.asarray(Wq, np.float32).astype(BF16)
    Wkb = np.asarray(Wk, np.float32).astype(BF16)
    Wvb = np.asarray(Wv, np.float32).astype(BF16)
    bq = np.asarray(bq, np.float32)
    bk = np.asarray(bk, np.float32)
    bv = np.asarray(bv, np.float32)

    with_mask = bool(np.any(mask != 0.0))
    with_bq = bool(np.any(bq != 0.0))
    with_bk = bool(np.any(bk != 0.0))
    with_bv = bool(np.any(bv != 0.0))
    flags = (dist, with_mask, with_bq, with_bk, with_bv)

    in_maps = []
    for c in range(N_CORES):
        b, h = c // 2, c % 2
        xq = qi[b, h * QL:(h + 1) * QL, :].astype(BF16)       # [QL, D]
        xkv = kv[b].astype(BF16)                               # [S, D]
        if dist:
            xkvt = np.ascontiguousarray(xkv[h * QL:(h + 1) * QL, :].T)
            perm_kv = np.concatenate(
                [xkv[h * QL:(h + 1) * QL], xkv[(1 - h) * QL:(2 - h) * QL]])
        else:
            xkvt = np.ascontiguousarray(xkv.T)
            perm_kv = xkv
        m = {
            "xqt": np.ascontiguousarray(xq.T),                 # [D, QL]
            "xkvt": xkvt,
            "xkv": np.ascontiguousarray(perm_kv),              # [S, D]
            "wq": Wqb, "wk": Wkb, "wv": Wvb,
        }
        if with_bq:
            m["bq"] = np.ascontiguousarray(bq.reshape(8, 128).T)
        if with_bk:
            m["bk"] = np.ascontiguousarray(bk.reshape(8, 128).T)
        if with_bv:
            m["bv"] = bv.astype(BF16).reshape(1, D)
        if with_mask:
            mt = mask[b, h * QL:(h + 1) * QL, :].T * np.float32(np.sqrt(D))
            if dist:
                mt = np.concatenate(
                    [mt[h * QL:(h + 1) * QL], mt[(1 - h) * QL:(2 - h) * QL]])
            m["maskt"] = np.ascontiguousarray(mt.astype(np.float32))
        in_maps.append(m)
    return flags, in_maps


def _run(inputs, trace=False, **kw):
    from concourse import bass_utils
    dist = os.environ.get("KERNEL_DIST", "1") == "1"
    flags, in_maps = _prep_in_maps(**inputs, dist=dist)
    nc = _get_nc(flags)
    res = bass_utils.run_bass_kernel_spmd(
        nc, in_maps, core_ids=list(range(N_CORES)), trace=trace, **kw)
    out = np.empty((B, S, D), np.float32)
    for c in range(N_CORES):
        b, h = c // 2, c % 2
        out[b, h * QL:(h + 1) * QL, :] = res.results[c]["out"]
    return out, res


def kernel(**inputs) -> np.ndarray:
    out, _ = _run(inputs, trace=False)
    return out


# revision 19
# speedup vs baseline: 1.0649x; 1.0649x over previous
"""Trainium2 Bass kernel: single-head attention (B=4, S=2048, D=1024) on 8 NeuronCores.

Sharding: data-parallel over (batch, query-half): core c handles batch c//2,
query rows [c%2*1024, (c%2+1)*1024), and (dist mode) computes the K-projection
only for its own kv half; the pair {2b, 2b+1} exchanges K^T via a 2-rank
AllGather overlapped with the Q-projection and the local-half attention.

Math per core (all matmuls bf16, fp32 PSUM accumulation):
  QT[dk,q]  = Wq(lhsT) . XqT(rhs)                 (+bq)
  KT[dk,s]  = Wk(lhsT) . XkvT(rhs)                (+bk)   [local half, AG for rest]
  sT[s,q]   = KT-tiles(lhsT) . QT(rhs)            scores^T
  eT[s,q]   = exp(sT / sqrt(dk) [+ maskT])        ScalarE, PSUM->SBUF bf16
  sums[1,q] = ones(lhsT) . eT(rhs)                softmax denominators
  HT[dm,q]  = Xkv-tiles(lhsT) . eT(rhs)           H = probs_unnorm @ Xkv
  out[q,dv] = HT-tiles(lhsT) . Wv(rhs)  (+bv)     (probs@Xkv)@Wv == probs@(Xkv@Wv)
  out      *= 1/sums  (per-partition scale on ScalarE, fused with PSUM->SBUF copy)

dist mode details: the s axis lives in LOCAL-relative order on each core
(m-tiles 0..7 = own kv half, 8..15 = partner's). The host feeds xkv/maskt
pre-permuted to match. The AllGather output is rank-ordered (identical layout
on both cores), so the partner block is recovered SPMD-uniformly via the exact
bf16-bit identity  remote = g0 XOR g1 XOR local  on uint32 views (own block
round-trips bit-identically through the collective).
"""

import os
import numpy as np
import ml_dtypes

B, S, D = 4, 2048, 1024
N_CORES = 8
QL = S // 2  # query rows per core (1024)
BF16 = ml_dtypes.bfloat16

_cache: dict = {}


def _build(dist: bool, with_mask: bool, with_bq: bool, with_bk: bool,
           with_bv: bool):
    import concourse.bass as bass
    import concourse.mybir as mybir
    import concourse.tile as tile
    from concourse import bacc

    fp32 = mybir.dt.float32
    bf16 = mybir.dt.bfloat16
    uint32 = mybir.dt.uint32

    nc = bacc.Bacc("TRN2", target_bir_lowering=False, debug=False,
                   num_devices=N_CORES)

    KVL = QL if dist else S  # kv rows projected locally
    xqt_d = nc.dram_tensor("xqt", (D, QL), bf16, kind="ExternalInput")
    xkvt_d = nc.dram_tensor("xkvt", (D, KVL), bf16, kind="ExternalInput")
    xkv_d = nc.dram_tensor("xkv", (S, D), bf16, kind="ExternalInput")
    wq_d = nc.dram_tensor("wq", (D, D), bf16, kind="ExternalInput")
    wk_d = nc.dram_tensor("wk", (D, D), bf16, kind="ExternalInput")
    wv_d = nc.dram_tensor("wv", (D, D), bf16, kind="ExternalInput")
    if with_bq:
        bq_d = nc.dram_tensor("bq", (128, 8), fp32, kind="ExternalInput")
    if with_bk:
        bk_d = nc.dram_tensor("bk", (128, 8), fp32, kind="ExternalInput")
    if with_bv:
        bv_d = nc.dram_tensor("bv", (1, D), bf16, kind="ExternalInput")
    if with_mask:
        maskt_d = nc.dram_tensor("maskt", (S, QL), fp32, kind="ExternalInput")
    out_d = nc.dram_tensor("out", (QL, D), fp32, kind="ExternalOutput")

    if dist:
        cc_in = nc.dram_tensor("cc_in", (D, QL), bf16)
        cc_out = nc.dram_tensor("cc_out", (2 * D, QL), bf16)
        groups = [[2 * g, 2 * g + 1] for g in range(4)]

    NT_D = D // 128    # 8 tiles along d_model / d_key
    NT_S = S // 128    # 16 tiles along s
    NT_L = NT_S // 2   # 8 (local-half s tiles in dist mode)
    NQ = QL // 512     # 2 query chunks of 512
    NKV = KVL // 512   # local kv chunks
    NV = D // 512      # 2 dv chunks of 512
    SCALE = 1.0 / float(np.sqrt(D))

    with tile.TileContext(nc) as tc:
        with (
            tc.tile_pool(name="cons", bufs=1) as cons,
            tc.tile_pool(name="a2", bufs=4) as a2,
            tc.tile_pool(name="kt", bufs=1) as ktp,
            tc.tile_pool(name="et", bufs=1) as etp,
            tc.tile_pool(name="xq", bufs=2) as xqp,
            tc.tile_pool(name="xkvt", bufs=2 if dist else 3) as xkvtp,
            tc.tile_pool(name="xkv", bufs=2) as xkvp,
            tc.tile_pool(name="gch", bufs=2) as gchp,
            tc.tile_pool(name="outp", bufs=3) as outp,
            tc.tile_pool(name="mask", bufs=2) as maskp,
            tc.tile_pool(name="ps", bufs=4, space=bass.MemorySpace.PSUM) as psp,
            tc.tile_pool(name="pss", bufs=2, space=bass.MemorySpace.PSUM) as pssp,
            tc.tile_pool(name="pst", bufs=1, space=bass.MemorySpace.PSUM) as pstp,
        ):
            # ---- constants ----
            ones_col = cons.tile([128, 1], bf16, tag="ones_col")
            nc.gpsimd.memset(ones_col[:], 1.0)
            ident1 = cons.tile([1, 1], fp32, tag="ident1")
            nc.gpsimd.memset(ident1[:], 1.0)
            if with_bv:
                ones_row = cons.tile([1, 128], bf16, tag="ones_row")
                nc.gpsimd.memset(ones_row[:], 1.0)
                bv_sb = cons.tile([1, D], bf16, tag="bv")
                nc.sync.dma_start(bv_sb[:], bv_d.ap()[:])
            if with_bq:
                bq_sb = cons.tile([128, 8], fp32, tag="bq")
                nc.sync.dma_start(bq_sb[:], bq_d.ap()[:])
            if with_bk:
                bk_sb = cons.tile([128, 8], fp32, tag="bk")
                nc.sync.dma_start(bk_sb[:], bk_d.ap()[:])

            # ---- weight + first-activation loads; issue order matters: the
            # first PE group needs wk + xkvt chunk 0, so those DMAs go first.
            wk_sb = a2.tile([128, NT_D, D], bf16, tag="a2")
            for j in range(NT_D):
                nc.sync.dma_start(wk_sb[:, j, :],
                                  wk_d.ap()[j * 128:(j + 1) * 128, :])

            qt_sb = a2.tile([128, NT_D, QL], bf16, tag="a2")
            kt_sb = ktp.tile([128, NT_D, KVL], bf16, tag="kt")
            if dist:
                ktr_sb = ktp.tile([128, NT_D, QL], bf16, tag="ktr")
            et_sb = etp.tile([128, NT_S, QL], bf16, tag="et")

            def proj_group(ps, w_sb, x_ch, i):
                for j in range(NT_D):
                    nc.tensor.matmul(
                        ps[:], w_sb[:, j, i * 128:(i + 1) * 128], x_ch[:, j, :],
                        start=(j == 0), stop=(j == NT_D - 1))

            # ---- stage 1b: KT (local half in dist mode) -> kt_sb ----
            kx_chunks = []
            for n in range(NKV):
                xkvt_ch = xkvtp.tile([128, NT_D, 512], bf16, tag="xkvt")
                for j in range(NT_D):
                    nc.sync.dma_start(
                        xkvt_ch[:, j, :],
                        xkvt_d.ap()[j * 128:(j + 1) * 128,
                                    n * 512:(n + 1) * 512])
                kx_chunks.append(xkvt_ch)
            for i in range(NT_D):
                for n in range(NKV):
                    ps = psp.tile([128, 512], fp32, tag="ps")
                    proj_group(ps, wk_sb, kx_chunks[n], i)
                    if with_bk:
                        nc.scalar.activation(
                            kt_sb[:, i, n * 512:(n + 1) * 512], ps[:],
                            mybir.ActivationFunctionType.Identity,
                            bias=bk_sb[:, i:i + 1])
                    else:
                        nc.scalar.activation(
                            kt_sb[:, i, n * 512:(n + 1) * 512], ps[:],
                            mybir.ActivationFunctionType.Copy)
                if dist:
                    nc.sync.dma_start(
                        cc_in.ap()[i * 128:(i + 1) * 128, :],
                        kt_sb[:, i, 0:QL])

            if dist:
                nc.gpsimd.collective_compute(
                    "AllGather", mybir.AluOpType.bypass,
                    replica_groups=groups,
                    ins=[cc_in.ap()[:].opt()],
                    outs=[cc_out.ap()[:].opt()],
                )

            # ---- stage 1a: QT -> qt_sb ----
            wq_sb = a2.tile([128, NT_D, D], bf16, tag="a2")
            for j in range(NT_D):
                nc.sync.dma_start(wq_sb[:, j, :],
                                  wq_d.ap()[j * 128:(j + 1) * 128, :])
            for n in range(NQ):
                xq_ch = xqp.tile([128, NT_D, 512], bf16, tag="xq")
                for j in range(NT_D):
                    nc.sync.dma_start(
                        xq_ch[:, j, :],
                        xqt_d.ap()[j * 128:(j + 1) * 128, n * 512:(n + 1) * 512])
                for i in range(NT_D):
                    ps = psp.tile([128, 512], fp32, tag="ps")
                    proj_group(ps, wq_sb, xq_ch, i)
                    if with_bq:
                        nc.scalar.activation(
                            qt_sb[:, i, n * 512:(n + 1) * 512], ps[:],
                            mybir.ActivationFunctionType.Identity,
                            bias=bq_sb[:, i:i + 1])
                    else:
                        nc.scalar.activation(
                            qt_sb[:, i, n * 512:(n + 1) * 512], ps[:],
                            mybir.ActivationFunctionType.Copy)

            # wv load early: reuses wk's pool slot once the K projection ends,
            # so stage 4 never waits on this DMA
            wv_sb = a2.tile([128, NT_D, D], bf16, tag="a2")
            for j in range(NT_D):
                nc.sync.dma_start(wv_sb[:, j, :],
                                  wv_d.ap()[j * 128:(j + 1) * 128, :])

            if dist:
                # read the gathered pair back, recover the partner's block via
                # remote = g0 ^ g1 ^ local (exact bf16 bit identity) -> ktr_sb.
                # 2D XOR slices only (3D APs fall off the DVE fast path), in
                # ascending s order so the first remote score groups unblock
                # as early as possible.
                for c in range(4):
                    g_ch = gchp.tile([128, 2 * NT_D, 256], bf16, tag="gch")
                    nc.sync.dma_start(
                        g_ch[:],
                        cc_out.ap()[:, c * 256:(c + 1) * 256]
                        .rearrange("(g p) f -> p g f", p=128))
                    for i in range(NT_D):
                        nc.vector.tensor_tensor(
                            g_ch[:, i, :].bitcast(uint32),
                            g_ch[:, i, :].bitcast(uint32),
                            g_ch[:, NT_D + i, :].bitcast(uint32),
                            mybir.AluOpType.bitwise_xor)
                        nc.vector.tensor_tensor(
                            ktr_sb[:, i, c * 256:(c + 1) * 256].bitcast(uint32),
                            g_ch[:, i, :].bitcast(uint32),
                            kt_sb[:, i, c * 256:c * 256 + 256].bitcast(uint32),
                            mybir.AluOpType.bitwise_xor)

            # ---- stage 2: scores^T + exp ----
            def score_group(m, n):
                kt, mm = (ktr_sb, m - NT_L) if (dist and m >= NT_L) else (kt_sb, m)
                ps = psp.tile([128, 512], fp32, tag="ps")
                for i in range(NT_D):
                    nc.tensor.matmul(
                        ps[:], kt[:, i, mm * 128:(mm + 1) * 128],
                        qt_sb[:, i, n * 512:(n + 1) * 512],
                        start=(i == 0), stop=(i == NT_D - 1))
                if with_mask:
                    mk = maskp.tile([128, 512], fp32, tag="mask")
                    nc.sync.dma_start(
                        mk[:], maskt_d.ap()[m * 128:(m + 1) * 128,
                                            n * 512:(n + 1) * 512])
                    nc.vector.tensor_tensor(
                        ps[:], ps[:], mk[:], mybir.AluOpType.add)
                nc.scalar.activation(
                    et_sb[:, m, n * 512:(n + 1) * 512], ps[:],
                    mybir.ActivationFunctionType.Exp, scale=SCALE)

            first_ms = range(NT_L) if dist else range(NT_S)
            for n in range(NQ):
                for m in first_ms:
                    score_group(m, n)

            # ---- stage 3a: HT over available s-tiles ----
            ht_sb = a2.tile([128, NT_D, QL], bf16, tag="a2")
            xkv_chunks = {}

            def ht_groups(ms, merge):
                for j in range(NT_D):
                    if (j, ms[0]) not in xkv_chunks:
                        xkv_ch = xkvp.tile([128, len(ms), 128], bf16, tag="xkv")
                        nc.sync.dma_start(
                            xkv_ch[:],
                            xkv_d.ap()[ms[0] * 128:(ms[-1] + 1) * 128,
                                       j * 128:(j + 1) * 128]
                            .rearrange("(m p) d -> p m d", p=128))
                        xkv_chunks[(j, ms[0])] = xkv_ch
                    xkv_ch = xkv_chunks[(j, ms[0])]
                    for n in range(NQ):
                        ps = psp.tile([128, 512], fp32, tag="ps")
                        for k, m in enumerate(ms):
                            nc.tensor.matmul(
                                ps[:], xkv_ch[:, k, :],
                                et_sb[:, m, n * 512:(n + 1) * 512],
                                start=(k == 0), stop=(k == len(ms) - 1))
                        dst = ht_sb[:, j, n * 512:(n + 1) * 512]
                        if merge:
                            nc.vector.tensor_tensor(
                                dst, ps[:], dst, mybir.AluOpType.add)
                        else:
                            nc.scalar.activation(
                                dst, ps[:],
                                mybir.ActivationFunctionType.Copy)

            if dist:
                ht_groups(list(range(NT_L)), merge=False)
                for n in range(NQ):
                    for m in range(NT_L, NT_S):
                        score_group(m, n)
            else:
                ht_groups(list(range(NT_S)), merge=False)

            # sums[1, q] = sum_s eT (all s-tiles)
            sums_sb = cons.tile([1, QL], fp32, tag="sums")
            for n in range(NQ):
                pss = pssp.tile([1, 512], fp32, tag="pss")
                for m in range(NT_S):
                    nc.tensor.matmul(
                        pss[:], ones_col[:], et_sb[:, m, n * 512:(n + 1) * 512],
                        start=(m == 0), stop=(m == NT_S - 1))
                nc.scalar.activation(
                    sums_sb[:, n * 512:(n + 1) * 512], pss[:],
                    mybir.ActivationFunctionType.Copy)

            if dist:
                ht_groups(list(range(NT_L, NT_S)), merge=True)

            # transpose sums -> [q(part), 1]; reciprocal
            pst = pstp.tile([128, 8], fp32, tag="pst")
            for p in range(8):
                nc.tensor.transpose(
                    pst[:, p:p + 1], sums_sb[:, p * 128:(p + 1) * 128],
                    ident1[:])
            recip_sb = cons.tile([128, 8], fp32, tag="recip")
            nc.vector.reciprocal(recip_sb[:], pst[:])

            # ---- stage 4: out = HT^T . Wv (+bv), normalized ----
            for p in range(8):
                out_sb = outp.tile([128, D], fp32, tag="outsb")
                for n2 in range(NV):
                    ps = psp.tile([128, 512], fp32, tag="ps")
                    for j in range(NT_D):
                        nc.tensor.matmul(
                            ps[:], ht_sb[:, j, p * 128:(p + 1) * 128],
                            wv_sb[:, j, n2 * 512:(n2 + 1) * 512],
                            start=(j == 0),
                            stop=(j == NT_D - 1 and not with_bv))
                    if with_bv:
                        nc.tensor.matmul(
                            ps[:], ones_row[:],
                            bv_sb[:, n2 * 512:(n2 + 1) * 512],
                            start=False, stop=True)
                    nc.scalar.activation(
                        out_sb[:, n2 * 512:(n2 + 1) * 512], ps[:],
                        mybir.ActivationFunctionType.Copy,
                        scale=recip_sb[:, p:p + 1])
                nc.sync.dma_start(
                    out_d.ap()[p * 128:(p + 1) * 128, :], out_sb[:])

    nc.compile()
    return nc


def _get_nc(flags):
    if flags not in _cache:
        _cache[flags] = _build(*flags)
    return _cache[flags]


def _prep_in_maps(query_input, keyvalue_input, mask, Wq, bq, Wk, bk, Wv, bv,
                  dist=True):
    qi = np.asarray(query_input, np.float32)
    kv = np.asarray(keyvalue_input, np.float32)
    mask = np.asarray(mask, np.float32)
    Wqb = np.asarray(Wq, np.float32).astype(BF16)
    Wkb = np.asarray(Wk, np.float32).astype(BF16)
    Wvb = np.asarray(Wv, np.float32).astype(BF16)
    bq = np.asarray(bq, np.float32)
    bk = np.asarray(bk, np.float32)
    bv = np.asarray(bv, np.float32)

    with_mask = bool(np.any(mask != 0.0))
    with_bq = bool(np.any(bq != 0.0))
    with_bk = bool(np.any(bk != 0.0))
    with_bv = bool(np.any(bv != 0.0))
    flags = (dist, with_mask, with_bq, with_bk, with_bv)

    in_maps = []
    for c in range(N_CORES):
        b, h = c // 2, c % 2
        xq = qi[b, h * QL:(h + 1) * QL, :].astype(BF16)       # [QL, D]
        xkv = kv[b].astype(BF16)                               # [S, D]
        if dist:
            xkvt = np.ascontiguousarray(xkv[h * QL:(h + 1) * QL, :].T)
            perm_kv = np.concatenate(
                [xkv[h * QL:(h + 1) * QL], xkv[(1 - h) * QL:(2 - h) * QL]])
        else:
            xkvt = np.ascontiguousarray(xkv.T)
            perm_kv = xkv
        m = {
            "xqt": np.ascontiguousarray(xq.T),                 # [D, QL]
            "xkvt": xkvt,
            "xkv": np.ascontiguousarray(perm_kv),              # [S, D]
            "wq": Wqb, "wk": Wkb, "wv": Wvb,
        }
        if with_bq:
            m["bq"] = np.ascontiguousarray(bq.reshape(8, 128).T)
        if with_bk:
            m["bk"] = np.ascontiguousarray(bk.reshape(8, 128).T)
        if with_bv:
            m["bv"] = bv.astype(BF16).reshape(1, D)
        if with_mask:
            mt = mask[b, h * QL:(h + 1) * QL, :].T * np.float32(np.sqrt(D))
            if dist:
                mt = np.concatenate(
                    [mt[h * QL:(h + 1) * QL], mt[(1 - h) * QL:(2 - h) * QL]])
            m["maskt"] = np.ascontiguousarray(mt.astype(np.float32))
        in_maps.append(m)
    return flags, in_maps


def _run(inputs, trace=False, **kw):
    from concourse import bass_utils
    dist = os.environ.get("KERNEL_DIST", "1") == "1"
    flags, in_maps = _prep_in_maps(**inputs, dist=dist)
    nc = _get_nc(flags)
    res = bass_utils.run_bass_kernel_spmd(
        nc, in_maps, core_ids=list(range(N_CORES)), trace=trace, **kw)
    out = np.empty((B, S, D), np.float32)
    for c in range(N_CORES):
        b, h = c // 2, c % 2
        out[b, h * QL:(h + 1) * QL, :] = res.results[c]["out"]
    return out, res


def kernel(**inputs) -> np.ndarray:
    out, _ = _run(inputs, trace=False)
    return out


# revision 21
# speedup vs baseline: 1.0925x; 1.0259x over previous
"""Trainium2 Bass kernel: single-head attention (B=4, S=2048, D=1024) on 8 NeuronCores.

Sharding: data-parallel over (batch, query-half): core c handles batch c//2,
query rows [c%2*1024, (c%2+1)*1024), and (dist mode) computes the K-projection
only for its own kv half; the pair {2b, 2b+1} exchanges K^T via a 2-rank
AllGather overlapped with the Q-projection and the local-half attention.

Math per core (all matmuls bf16, fp32 PSUM accumulation):
  QT[dk,q]  = Wq(lhsT) . XqT(rhs)                 (+bq)
  KT[dk,s]  = Wk(lhsT) . XkvT(rhs)                (+bk)   [local half, AG for rest]
  sT[s,q]   = KT-tiles(lhsT) . QT(rhs)            scores^T
  eT[s,q]   = exp(sT / sqrt(dk) [+ maskT])        ScalarE, PSUM->SBUF bf16
  sums[1,q] = ones(lhsT) . eT(rhs)                softmax denominators
  HT[dm,q]  = Xkv-tiles(lhsT) . eT(rhs)           H = probs_unnorm @ Xkv
  out[q,dv] = HT-tiles(lhsT) . Wv(rhs)  (+bv)     (probs@Xkv)@Wv == probs@(Xkv@Wv)
  out      *= 1/sums  (per-partition scale on ScalarE, fused with PSUM->SBUF copy)

dist mode details: the s axis lives in LOCAL-relative order on each core
(m-tiles 0..7 = own kv half, 8..15 = partner's). The host feeds xkv/maskt
pre-permuted to match. The AllGather output is rank-ordered (identical layout
on both cores), so the partner block is recovered SPMD-uniformly via the exact
bf16-bit identity  remote = g0 XOR g1 XOR local  on uint32 views (own block
round-trips bit-identically through the collective).
"""

import os
import numpy as np
import ml_dtypes

B, S, D = 4, 2048, 1024
N_CORES = 8
QL = S // 2  # query rows per core (1024)
BF16 = ml_dtypes.bfloat16

_cache: dict = {}


def _build(dist: bool, with_mask: bool, with_bq: bool, with_bk: bool,
           with_bv: bool):
    import concourse.bass as bass
    import concourse.mybir as mybir
    import concourse.tile as tile
    from concourse import bacc

    fp32 = mybir.dt.float32
    bf16 = mybir.dt.bfloat16
    uint32 = mybir.dt.uint32

    nc = bacc.Bacc("TRN2", target_bir_lowering=False, debug=False,
                   num_devices=N_CORES)

    KVL = QL if dist else S  # kv rows projected locally
    xqt_d = nc.dram_tensor("xqt", (D, QL), bf16, kind="ExternalInput")
    xkvt_d = nc.dram_tensor("xkvt", (D, KVL), bf16, kind="ExternalInput")
    xkv_d = nc.dram_tensor("xkv", (S, D), bf16, kind="ExternalInput")
    wq_d = nc.dram_tensor("wq", (D, D), bf16, kind="ExternalInput")
    wk_d = nc.dram_tensor("wk", (D, D), bf16, kind="ExternalInput")
    wv_d = nc.dram_tensor("wv", (D, D), bf16, kind="ExternalInput")
    if with_bq:
        bq_d = nc.dram_tensor("bq", (128, 8), fp32, kind="ExternalInput")
    if with_bk:
        bk_d = nc.dram_tensor("bk", (128, 8), fp32, kind="ExternalInput")
    if with_bv:
        bv_d = nc.dram_tensor("bv", (1, D), bf16, kind="ExternalInput")
    if with_mask:
        maskt_d = nc.dram_tensor("maskt", (S, QL), fp32, kind="ExternalInput")
    out_d = nc.dram_tensor("out", (QL, D), fp32, kind="ExternalOutput")

    if dist:
        cc_in = nc.dram_tensor("cc_in", (D, QL), bf16)
        cc_out = nc.dram_tensor("cc_out", (2 * D, QL), bf16)
        groups = [[2 * g, 2 * g + 1] for g in range(4)]

    NT_D = D // 128    # 8 tiles along d_model / d_key
    NT_S = S // 128    # 16 tiles along s
    NT_L = NT_S // 2   # 8 (local-half s tiles in dist mode)
    NQ = QL // 512     # 2 query chunks of 512
    NKV = KVL // 512   # local kv chunks
    NV = D // 512      # 2 dv chunks of 512
    SCALE = 1.0 / float(np.sqrt(D))

    with tile.TileContext(nc) as tc:
        with (
            tc.tile_pool(name="cons", bufs=1) as cons,
            tc.tile_pool(name="a2", bufs=4) as a2,
            tc.tile_pool(name="kt", bufs=1) as ktp,
            tc.tile_pool(name="et", bufs=1) as etp,
            tc.tile_pool(name="xq", bufs=2) as xqp,
            tc.tile_pool(name="xkvt", bufs=2 if dist else 3) as xkvtp,
            tc.tile_pool(name="xkv", bufs=2) as xkvp,
            tc.tile_pool(name="gch", bufs=4) as gchp,
            tc.tile_pool(name="outp", bufs=3) as outp,
            tc.tile_pool(name="mask", bufs=2) as maskp,
            tc.tile_pool(name="ps", bufs=4, space=bass.MemorySpace.PSUM) as psp,
            tc.tile_pool(name="pss", bufs=2, space=bass.MemorySpace.PSUM) as pssp,
            tc.tile_pool(name="pst", bufs=1, space=bass.MemorySpace.PSUM) as pstp,
        ):
            # ---- constants ----
            ones_col = cons.tile([128, 1], bf16, tag="ones_col")
            nc.gpsimd.memset(ones_col[:], 1.0)
            ident1 = cons.tile([1, 1], fp32, tag="ident1")
            nc.gpsimd.memset(ident1[:], 1.0)
            if with_bv:
                ones_row = cons.tile([1, 128], bf16, tag="ones_row")
                nc.gpsimd.memset(ones_row[:], 1.0)
                bv_sb = cons.tile([1, D], bf16, tag="bv")
                nc.sync.dma_start(bv_sb[:], bv_d.ap()[:])
            if with_bq:
                bq_sb = cons.tile([128, 8], fp32, tag="bq")
                nc.sync.dma_start(bq_sb[:], bq_d.ap()[:])
            if with_bk:
                bk_sb = cons.tile([128, 8], fp32, tag="bk")
                nc.sync.dma_start(bk_sb[:], bk_d.ap()[:])

            # ---- weight + first-activation loads; issue order matters: the
            # first PE group needs wk + xkvt chunk 0, so those DMAs go first.
            wk_sb = a2.tile([128, NT_D, D], bf16, tag="a2")
            for j in range(NT_D):
                nc.sync.dma_start(wk_sb[:, j, :],
                                  wk_d.ap()[j * 128:(j + 1) * 128, :])

            qt_sb = a2.tile([128, NT_D, QL], bf16, tag="a2")
            kt_sb = ktp.tile([128, NT_D, KVL], bf16, tag="kt")
            if dist:
                ktr_sb = ktp.tile([128, NT_D, QL], bf16, tag="ktr")
            et_sb = etp.tile([128, NT_S, QL], bf16, tag="et")

            def proj_group(ps, w_sb, x_ch, i):
                for j in range(NT_D):
                    nc.tensor.matmul(
                        ps[:], w_sb[:, j, i * 128:(i + 1) * 128], x_ch[:, j, :],
                        start=(j == 0), stop=(j == NT_D - 1))

            # ---- stage 1b: KT (local half in dist mode) -> kt_sb ----
            kx_chunks = []
            for n in range(NKV):
                xkvt_ch = xkvtp.tile([128, NT_D, 512], bf16, tag="xkvt")
                for j in range(NT_D):
                    nc.sync.dma_start(
                        xkvt_ch[:, j, :],
                        xkvt_d.ap()[j * 128:(j + 1) * 128,
                                    n * 512:(n + 1) * 512])
                kx_chunks.append(xkvt_ch)
            for i in range(NT_D):
                for n in range(NKV):
                    ps = psp.tile([128, 512], fp32, tag="ps")
                    proj_group(ps, wk_sb, kx_chunks[n], i)
                    if with_bk:
                        nc.scalar.activation(
                            kt_sb[:, i, n * 512:(n + 1) * 512], ps[:],
                            mybir.ActivationFunctionType.Identity,
                            bias=bk_sb[:, i:i + 1])
                    else:
                        nc.scalar.activation(
                            kt_sb[:, i, n * 512:(n + 1) * 512], ps[:],
                            mybir.ActivationFunctionType.Copy)
                if dist:
                    nc.sync.dma_start(
                        cc_in.ap()[i * 128:(i + 1) * 128, :],
                        kt_sb[:, i, 0:QL])

            if dist:
                nc.gpsimd.collective_compute(
                    "AllGather", mybir.AluOpType.bypass,
                    replica_groups=groups,
                    ins=[cc_in.ap()[:].opt()],
                    outs=[cc_out.ap()[:].opt()],
                )

            # ---- stage 1a: QT -> qt_sb ----
            wq_sb = a2.tile([128, NT_D, D], bf16, tag="a2")
            for j in range(NT_D):
                nc.sync.dma_start(wq_sb[:, j, :],
                                  wq_d.ap()[j * 128:(j + 1) * 128, :])
            for n in range(NQ):
                xq_ch = xqp.tile([128, NT_D, 512], bf16, tag="xq")
                for j in range(NT_D):
                    nc.sync.dma_start(
                        xq_ch[:, j, :],
                        xqt_d.ap()[j * 128:(j + 1) * 128, n * 512:(n + 1) * 512])
                for i in range(NT_D):
                    ps = psp.tile([128, 512], fp32, tag="ps")
                    proj_group(ps, wq_sb, xq_ch, i)
                    if with_bq:
                        nc.scalar.activation(
                            qt_sb[:, i, n * 512:(n + 1) * 512], ps[:],
                            mybir.ActivationFunctionType.Identity,
                            bias=bq_sb[:, i:i + 1])
                    else:
                        nc.scalar.activation(
                            qt_sb[:, i, n * 512:(n + 1) * 512], ps[:],
                            mybir.ActivationFunctionType.Copy)

            # wv load early: reuses wk's pool slot once the K projection ends,
            # so stage 4 never waits on this DMA
            wv_sb = a2.tile([128, NT_D, D], bf16, tag="a2")
            for j in range(NT_D):
                nc.sync.dma_start(wv_sb[:, j, :],
                                  wv_d.ap()[j * 128:(j + 1) * 128, :])

            if dist:
                # read the gathered pair back, recover the partner's block via
                # remote = g0 ^ g1 ^ local (exact bf16 bit identity) -> ktr_sb.
                # per-i readback: fully contiguous 2KB DMA segments, 2D XORs
                cc_view = cc_out.ap().rearrange("(b r) f -> r b f", b=2)
                for i in range(NT_D):
                    g_ch = gchp.tile([128, 2, QL], bf16, tag="gch")
                    nc.sync.dma_start(
                        g_ch[:], cc_view[i * 128:(i + 1) * 128, :, :])
                    nc.vector.tensor_tensor(
                        g_ch[:, 0, :].bitcast(uint32),
                        g_ch[:, 0, :].bitcast(uint32),
                        g_ch[:, 1, :].bitcast(uint32),
                        mybir.AluOpType.bitwise_xor)
                    nc.vector.tensor_tensor(
                        ktr_sb[:, i, :].bitcast(uint32),
                        g_ch[:, 0, :].bitcast(uint32),
                        kt_sb[:, i, :].bitcast(uint32),
                        mybir.AluOpType.bitwise_xor)

            # ---- stage 2: scores^T + exp ----
            def score_group(m, n):
                kt, mm = (ktr_sb, m - NT_L) if (dist and m >= NT_L) else (kt_sb, m)
                ps = psp.tile([128, 512], fp32, tag="ps")
                for i in range(NT_D):
                    nc.tensor.matmul(
                        ps[:], kt[:, i, mm * 128:(mm + 1) * 128],
                        qt_sb[:, i, n * 512:(n + 1) * 512],
                        start=(i == 0), stop=(i == NT_D - 1))
                if with_mask:
                    mk = maskp.tile([128, 512], fp32, tag="mask")
                    nc.sync.dma_start(
                        mk[:], maskt_d.ap()[m * 128:(m + 1) * 128,
                                            n * 512:(n + 1) * 512])
                    nc.vector.tensor_tensor(
                        ps[:], ps[:], mk[:], mybir.AluOpType.add)
                nc.scalar.activation(
                    et_sb[:, m, n * 512:(n + 1) * 512], ps[:],
                    mybir.ActivationFunctionType.Exp, scale=SCALE)

            first_ms = range(NT_L) if dist else range(NT_S)
            for n in range(NQ):
                for m in first_ms:
                    score_group(m, n)

            # ---- stage 3a: HT over available s-tiles ----
            ht_sb = a2.tile([128, NT_D, QL], bf16, tag="a2")
            xkv_chunks = {}

            def ht_groups(ms, merge):
                for j in range(NT_D):
                    if (j, ms[0]) not in xkv_chunks:
                        xkv_ch = xkvp.tile([128, len(ms), 128], bf16, tag="xkv")
                        nc.sync.dma_start(
                            xkv_ch[:],
                            xkv_d.ap()[ms[0] * 128:(ms[-1] + 1) * 128,
                                       j * 128:(j + 1) * 128]
                            .rearrange("(m p) d -> p m d", p=128))
                        xkv_chunks[(j, ms[0])] = xkv_ch
                    xkv_ch = xkv_chunks[(j, ms[0])]
                    for n in range(NQ):
                        ps = psp.tile([128, 512], fp32, tag="ps")
                        for k, m in enumerate(ms):
                            nc.tensor.matmul(
                                ps[:], xkv_ch[:, k, :],
                                et_sb[:, m, n * 512:(n + 1) * 512],
                                start=(k == 0), stop=(k == len(ms) - 1))
                        dst = ht_sb[:, j, n * 512:(n + 1) * 512]
                        if merge:
                            nc.vector.tensor_tensor(
                                dst, ps[:], dst, mybir.AluOpType.add)
                        else:
                            nc.scalar.activation(
                                dst, ps[:],
                                mybir.ActivationFunctionType.Copy)

            if dist:
                ht_groups(list(range(NT_L)), merge=False)
                for n in range(NQ):
                    for m in range(NT_L, NT_S):
                        score_group(m, n)
            else:
                ht_groups(list(range(NT_S)), merge=False)

            # sums[1, q] = sum_s eT (all s-tiles)
            sums_sb = cons.tile([1, QL], fp32, tag="sums")
            for n in range(NQ):
                pss = pssp.tile([1, 512], fp32, tag="pss")
                for m in range(NT_S):
                    nc.tensor.matmul(
                        pss[:], ones_col[:], et_sb[:, m, n * 512:(n + 1) * 512],
                        start=(m == 0), stop=(m == NT_S - 1))
                nc.scalar.activation(
                    sums_sb[:, n * 512:(n + 1) * 512], pss[:],
                    mybir.ActivationFunctionType.Copy)

            if dist:
                ht_groups(list(range(NT_L, NT_S)), merge=True)

            # transpose sums -> [q(part), 1]; reciprocal
            pst = pstp.tile([128, 8], fp32, tag="pst")
            for p in range(8):
                nc.tensor.transpose(
                    pst[:, p:p + 1], sums_sb[:, p * 128:(p + 1) * 128],
                    ident1[:])
            recip_sb = cons.tile([128, 8], fp32, tag="recip")
            nc.vector.reciprocal(recip_sb[:], pst[:])

            # ---- stage 4: out = HT^T . Wv (+bv), normalized ----
            for p in range(8):
                out_sb = outp.tile([128, D], fp32, tag="outsb")
                for n2 in range(NV):
                    ps = psp.tile([128, 512], fp32, tag="ps")
                    for j in range(NT_D):
                        nc.tensor.matmul(
                            ps[:], ht_sb[:, j, p * 128:(p + 1) * 128],
                            wv_sb[:, j, n2 * 512:(n2 + 1) * 512],
                            start=(j == 0),
                            stop=(j == NT_D - 1 and not with_bv))
                    if with_bv:
                        nc.tensor.matmul(
                            ps[:], ones_row[:],
                            bv_sb[:, n2 * 512:(n2 + 1) * 512],
                            start=False, stop=True)
                    nc.scalar.activation(
                        out_sb[:, n2 * 512:(n2 + 1) * 512], ps[:],
                        mybir.ActivationFunctionType.Copy,
                        scale=recip_sb[:, p:p + 1])
                nc.sync.dma_start(
                    out_d.ap()[p * 128:(p + 1) * 128, :], out_sb[:])

    nc.compile()
    return nc


def _get_nc(flags):
    if flags not in _cache:
        _cache[flags] = _build(*flags)
    return _cache[flags]


def _prep_in_maps(query_input, keyvalue_input, mask, Wq, bq, Wk, bk, Wv, bv,
                  dist=True):
    qi = np.asarray(query_input, np.float32)
    kv = np.asarray(keyvalue_input, np.float32)
    mask = np.asarray(mask, np.float32)
    Wqb = np.asarray(Wq, np.float32).astype(BF16)
    Wkb = np.asarray(Wk, np.float32).astype(BF16)
    Wvb = np.asarray(Wv, np.float32).astype(BF16)
    bq = np.asarray(bq, np.float32)
    bk = np.asarray(bk, np.float32)
    bv = np.asarray(bv, np.float32)

    with_mask = bool(np.any(mask != 0.0))
    with_bq = bool(np.any(bq != 0.0))
    with_bk = bool(np.any(bk != 0.0))
    with_bv = bool(np.any(bv != 0.0))
    flags = (dist, with_mask, with_bq, with_bk, with_bv)

    in_maps = []
    for c in range(N_CORES):
        b, h = c // 2, c % 2
        xq = qi[b, h * QL:(h + 1) * QL, :].astype(BF16)       # [QL, D]
        xkv = kv[b].astype(BF16)                               # [S, D]
        if dist:
            xkvt = np.ascontiguousarray(xkv[h * QL:(h + 1) * QL, :].T)
            perm_kv = np.concatenate(
                [xkv[h * QL:(h + 1) * QL], xkv[(1 - h) * QL:(2 - h) * QL]])
        else:
            xkvt = np.ascontiguousarray(xkv.T)
            perm_kv = xkv
        m = {
            "xqt": np.ascontiguousarray(xq.T),                 # [D, QL]
            "xkvt": xkvt,
            "xkv": np.ascontiguousarray(perm_kv),              # [S, D]
            "wq": Wqb, "wk": Wkb, "wv": Wvb,
        }
        if with_bq:
            m["bq"] = np.ascontiguousarray(bq.reshape(8, 128).T)
        if with_bk:
            m["bk"] = np.ascontiguousarray(bk.reshape(8, 128).T)
        if with_bv:
            m["bv"] = bv.astype(BF16).reshape(1, D)
        if with_mask:
            mt = mask[b, h * QL:(h + 1) * QL, :].T * np.float32(np.sqrt(D))
            if dist:
                mt = np.concatenate(
                    [mt[h * QL:(h + 1) * QL], mt[(1 - h) * QL:(2 - h) * QL]])
            m["maskt"] = np.ascontiguousarray(mt.astype(np.float32))
        in_maps.append(m)
    return flags, in_maps


def _run(inputs, trace=False, **kw):
    from concourse import bass_utils
    dist = os.environ.get("KERNEL_DIST", "1") == "1"
    flags, in_maps = _prep_in_maps(**inputs, dist=dist)
    nc = _get_nc(flags)
    res = bass_utils.run_bass_kernel_spmd(
        nc, in_maps, core_ids=list(range(N_CORES)), trace=trace, **kw)
    out = np.empty((B, S, D), np.float32)
    for c in range(N_CORES):
        b, h = c // 2, c % 2
        out[b, h * QL:(h + 1) * QL, :] = res.results[c]["out"]
    return out, res


def kernel(**inputs) -> np.ndarray:
    out, _ = _run(inputs, trace=False)
    return out
